# revision 24
# baseline (speedup 1.0000x reference)
"""TRN2 Bass kernel for nn_EuclideanCodebook (VQ codebook with EMA update).

Strategy (data-parallel over N, 8 NeuronCores):
 - shard x [262144, 64] row-wise -> 32768 rows/core; replicate codebook consts
 - per 128-row tile: PE computes s = x @ (-2 e)^T (fp32, exact reference
   rounding order: dist = (x2 - 2s) + e2 reproduced via ACT bias + DVE add,
   negated end-to-end so DVE max8/max_index give argmin w/ first-tie like jnp)
 - onehot(argmin) built on GPSIMD; PE scatter-matmul accumulates
   [embed_sum | bins] in PSUM across all tiles
 - AllReduce bins/embed_sum across the 8 cores (gpsimd collective), then the
   EMA + Laplace-smoothing epilogue on-device
 - quantize = embed[embed_ind] via one dma_gather (idx relayout through 16
   PE transposes into the 16-partition-wrapped int16 index layout)
"""
import sys
sys.path.insert(0, '/opt/trn_rl_repo')
import numpy as np
import concourse.bass as bass
from concourse import bacc
import concourse.mybir as mybir
import concourse.tile as tile
from concourse.bass_utils import run_bass_kernel_spmd

dt = mybir.dt

N, D, K = 262144, 64, 1024
NCORES = 8
NL = N // NCORES          # 32768 rows per core
TPP = NL // 128           # 256 rows per partition (= tiles per core)
T = TPP                   # 256 tiles of 128 rows
KC = K // 128             # 8 onehot chunks
DECAY = 0.1
EPS = 1e-5

_cached = {}

# feature flags (bisection/testing)
DO_SCATTER = True
DO_CC = True
DO_GATHER = True
GATHER_CHUNK = 4096       # rows per dma_gather call
OH_ON_DVE = False         # onehot on DVE instead of GPSIMD
LINEARIZE = False
DEBUG_OHSUM = False


def _build_program():
    nc = bacc.Bacc("TRN2", target_bir_lowering=False, debug=False)

    # inputs
    d_x = nc.dram_tensor("x_loc", [NL, D], dt.float32, kind="ExternalInput")
    d_e2T = nc.dram_tensor("e2T", [D, K], dt.float32, kind="ExternalInput")      # (2*embed)^T
    d_e2neg = nc.dram_tensor("e2neg", [128, K], dt.float32, kind="ExternalInput")  # -|e|^2 replicated
    d_iota = nc.dram_tensor("iota", [128, K], dt.float32, kind="ExternalInput")
    d_ident = nc.dram_tensor("ident", [128, 128], dt.float32, kind="ExternalInput")
    d_etbl = nc.dram_tensor("etbl", [K, D], dt.float32, kind="ExternalInput")    # embed for gather
    d_cs = nc.dram_tensor("cs_r", [128, KC], dt.float32, kind="ExternalInput")   # cluster_size [p,c]
    d_ea = nc.dram_tensor("ea_r", [128, KC * D], dt.float32, kind="ExternalInput")  # embed_avg [p,c,:]

    # outputs
    o_q = nc.dram_tensor("q_r", [128, TPP * D], dt.float32, kind="ExternalOutput")
    o_ei = nc.dram_tensor("ei_r", [128, TPP], dt.int32, kind="ExternalOutput")
    o_csn = nc.dram_tensor("csn_r", [128, KC], dt.float32, kind="ExternalOutput")
    o_ean = nc.dram_tensor("ean_r", [128, KC * D], dt.float32, kind="ExternalOutput")
    o_enr = nc.dram_tensor("enr_r", [128, KC * D], dt.float32, kind="ExternalOutput")
    o_dbg = (nc.dram_tensor("dbg_r", [128, TPP], dt.float32, kind="ExternalOutput")
             if DEBUG_OHSUM else None)

    # collective bounce buffers
    cc_in = nc.dram_tensor("cc_in", [128, KC * (D + 1)], dt.float32)
    cc_out = nc.dram_tensor("cc_out", [128, KC * (D + 1)], dt.float32, addr_space="Shared")

    x_view = d_x.rearrange("(p t) d -> p t d", p=128)     # row n = p*TPP + t

    with tile.TileContext(nc, linearize=LINEARIZE) as tc:
        with tc.tile_pool(name="const", bufs=1) as cpool, \
             tc.tile_pool(name="xall", bufs=1) as xpool, \
             tc.tile_pool(name="acc", bufs=1) as apool, \
             tc.tile_pool(name="pacc", bufs=1, space="PSUM") as ppacc:

            # ---- resident tiles ----
            t_e2T = cpool.tile([D, K], dt.float32)
            nc.sync.dma_start(t_e2T[:], d_e2T[:])
            t_e2neg = cpool.tile([128, K], dt.float32)
            nc.sync.dma_start(t_e2neg[:], d_e2neg[:])
            t_iota = cpool.tile([128, K], dt.float32)
            nc.sync.dma_start(t_iota[:], d_iota[:])
            t_ident = cpool.tile([128, 128], dt.float32)
            nc.sync.dma_start(t_ident[:], d_ident[:])

            # x resident: [128, T, D+1] with a ones column at d=D
            t_xall = xpool.tile([128, T, D + 1], dt.float32)
            nc.sync.dma_start(t_xall[:, :, 0:D], x_view[:])
            nc.vector.memset(t_xall[:, :, D:D + 1], 1.0)

            # index accumulator (fp32, exact ints)
            t_idxf = apool.tile([128, T], dt.float32)
            t_ohsum = apool.tile([128, T], dt.float32, name="t_ohsum") if DEBUG_OHSUM else None

            # [embed_sum | bins] PSUM accumulator: chunk c at [:, c, 0:65]
            p_es = ppacc.tile([128, KC, 128], dt.float32)

            with tc.tile_pool(name="sb", bufs=3) as pool, \
                 tc.tile_pool(name="ps", bufs=2, space="PSUM") as pps, \
                 tc.tile_pool(name="psT", bufs=2, space="PSUM") as ppsT:
                for t in range(T):
                    xt = t_xall[:, t, 0:D]
                    rhs65 = t_xall[:, t, :]

                    # xT via PE transpose
                    p_xT = ppsT.tile([D, 128], dt.float32)
                    nc.tensor.transpose(p_xT[:], xt, t_ident[:])
                    t_xT = pool.tile([D, 128], dt.float32, tag="xT")
                    nc.scalar.copy(t_xT[:], p_xT[:])

                    # x2 via ACT square + accumulate, then negate
                    t_sq = pool.tile([128, D], dt.float32, tag="sq")
                    t_x2 = pool.tile([128, 1], dt.float32, tag="x2")
                    nc.scalar.activation(t_sq[:], xt,
                                         mybir.ActivationFunctionType.Square,
                                         accum_out=t_x2[:])
                    t_x2n = pool.tile([128, 1], dt.float32, tag="x2n")
                    nc.vector.tensor_scalar_mul(t_x2n[:], t_x2[:], -1.0)

                    # s' = x @ (2e)^T  [128, 1024] fp32 (2 PSUM banks)
                    p_s = pps.tile([128, K], dt.float32)
                    nc.tensor.matmul(p_s[:, 0:512], t_xT[:], t_e2T[:, 0:512],
                                     start=True, stop=True)
                    nc.tensor.matmul(p_s[:, 512:1024], t_xT[:], t_e2T[:, 512:1024],
                                     start=True, stop=True)

                    # tt = fl(s' - x2) = -fl(x2 - 2s)
                    t_t = pool.tile([128, K], dt.float32, tag="t")
                    nc.scalar.activation(t_t[:], p_s[:],
                                         mybir.ActivationFunctionType.Identity,
                                         bias=t_x2n[:], scale=1.0)
                    # d' = fl(tt - e2) = -dist  (split DVE/GPSIMD to balance)
                    t_d = pool.tile([128, K], dt.float32, tag="d")
                    SPL = 256
                    nc.vector.tensor_tensor(t_d[:, 0:SPL], t_t[:, 0:SPL],
                                            t_e2neg[:, 0:SPL], mybir.AluOpType.add)
                    nc.gpsimd.tensor_tensor(t_d[:, SPL:], t_t[:, SPL:],
                                            t_e2neg[:, SPL:], mybir.AluOpType.add)

                    # argmin via max8 + max_index (first-tie like jnp.argmin)
                    t_m8 = pool.tile([128, 8], dt.float32, tag="m8")
                    nc.vector.max(out=t_m8[:], in_=t_d[:])
                    t_i8 = pool.tile([128, 8], dt.uint16, tag="i8")
                    nc.vector.max_index(t_i8[:], t_m8[:], t_d[:])
                    nc.vector.tensor_copy(t_idxf[:, t:t + 1], t_i8[:, 0:1])

                    # onehot
                    t_oh = pool.tile([128, K], dt.float32, tag="oh")
                    oh_eng = nc.vector if OH_ON_DVE else nc.gpsimd
                    oh_eng.tensor_scalar(t_oh[:], t_iota[:],
                                         t_idxf[:, t:t + 1], None,
                                         mybir.AluOpType.is_equal)

                    if DEBUG_OHSUM:
                        nc.vector.tensor_reduce(t_ohsum[:, t:t + 1], t_oh[:],
                                                mybir.AxisListType.X,
                                                mybir.AluOpType.add)

                    # scatter-accumulate [embed_sum | bins]
                    if DO_SCATTER:
                        # start=True clears has_written for the WHOLE 2KB bank,
                        # so only the first chunk of each bank may set it.
                        for c in range(KC):
                            nc.tensor.matmul(p_es[:, c, 0:D + 1],
                                             t_oh[:, c * 128:(c + 1) * 128], rhs65,
                                             start=(t == 0 and c % 4 == 0),
                                             stop=(t == T - 1),
                                             skip_group_check=True)

            # ================= epilogue =================
            with tc.tile_pool(name="ep", bufs=1) as ep, \
                 tc.tile_pool(name="eps", bufs=1, space="PSUM") as epps:
                # embed_sum partial -> SBUF -> AllReduce
                t_es = ep.tile([128, KC, D + 1], dt.float32)
                if DO_SCATTER:
                    nc.scalar.copy(t_es[:], p_es[:, :, 0:D + 1])
                else:
                    nc.vector.memset(t_es[:], 1.0)
                t_esg = ep.tile([128, KC, D + 1], dt.float32)
                if DO_CC:
                    nc.sync.dma_start(cc_in[:], t_es[:].rearrange("p c k -> p (c k)"))
                    nc.gpsimd.collective_compute(
                        "AllReduce", mybir.AluOpType.add,
                        replica_groups=[list(range(NCORES))],
                        ins=[cc_in[:]], outs=[cc_out[:]])
                    nc.sync.dma_start(t_esg[:].rearrange("p c k -> p (c k)"), cc_out[:])
                else:
                    nc.vector.tensor_copy(t_esg[:], t_es[:])

                if DEBUG_OHSUM:
                    nc.sync.dma_start(o_dbg[:], t_ohsum[:])
                # embed_ind int32 out
                t_ei = ep.tile([128, T], dt.int32)
                nc.vector.tensor_copy(t_ei[:], t_idxf[:])
                nc.sync.dma_start(o_ei[:], t_ei[:])

                # ---- EMA update ----
                t_cs = ep.tile([128, KC], dt.float32)
                nc.sync.dma_start(t_cs[:], d_cs[:])
                t_ea = ep.tile([128, KC, D], dt.float32)
                nc.sync.dma_start(t_ea[:].rearrange("p c d -> p (c d)"), d_ea[:])

                bins = t_esg[:, :, D:D + 1]          # [128, KC, 1]
                esum = t_esg[:, :, 0:D]              # [128, KC, D]

                # cluster_size_new = cs*DECAY + bins*(1-DECAY)
                t_csd = ep.tile([128, KC], dt.float32)
                nc.vector.tensor_scalar_mul(t_csd[:], t_cs[:], DECAY)
                t_bnd = ep.tile([128, KC], dt.float32)
                nc.vector.tensor_scalar_mul(
                    t_bnd[:], bins.rearrange("p c one -> p (c one)"), 1.0 - DECAY)
                t_csn = ep.tile([128, KC], dt.float32)
                nc.vector.tensor_tensor(t_csn[:], t_csd[:], t_bnd[:],
                                        mybir.AluOpType.add)
                nc.sync.dma_start(o_csn[:], t_csn[:])

                # embed_avg_new = ea*DECAY + esum*(1-DECAY)
                t_ead = ep.tile([128, KC, D], dt.float32)
                nc.vector.tensor_scalar_mul(t_ead[:], t_ea[:], DECAY)
                t_esd = ep.tile([128, KC, D], dt.float32)
                nc.vector.tensor_scalar_mul(t_esd[:], esum, 1.0 - DECAY)
                t_ean = ep.tile([128, KC, D], dt.float32)
                nc.vector.tensor_tensor(t_ean[:], t_ead[:], t_esd[:],
                                        mybir.AluOpType.add)
                nc.sync.dma_start(o_ean[:], t_ean[:].rearrange("p c d -> p (c d)"))

                # n = sum(cluster_size_new) -> broadcast to all partitions
                t_ones = ep.tile([128, 1], dt.float32)
                nc.vector.memset(t_ones[:], 1.0)
                t_csp = ep.tile([128, 1], dt.float32)
                nc.vector.tensor_reduce(t_csp[:], t_csn[:], mybir.AxisListType.X,
                                        mybir.AluOpType.add)
                p_n = epps.tile([1, 1], dt.float32)
                nc.tensor.matmul(p_n[:], t_csp[:], t_ones[:],
                                 start=True, stop=True)
                t_n1 = ep.tile([1, 1], dt.float32)
                nc.scalar.copy(t_n1[:], p_n[:])
                t_ones_row = ep.tile([1, 128], dt.float32)
                nc.vector.memset(t_ones_row[:], 1.0)
                p_nb = epps.tile([128, 1], dt.float32)
                nc.tensor.matmul(p_nb[:], t_ones_row[:], t_n1[:],
                                 start=True, stop=True)
                t_nb = ep.tile([128, 1], dt.float32)
                nc.scalar.copy(t_nb[:], p_nb[:])

                # cs_smoothed = (csn + EPS) / (n + K*EPS) * n
                t_cse = ep.tile([128, KC], dt.float32)
                nc.vector.tensor_scalar_add(t_cse[:], t_csn[:], EPS)
                t_ne = ep.tile([128, 1], dt.float32)
                nc.vector.tensor_scalar_add(t_ne[:], t_nb[:], float(K * EPS))
                t_rne = ep.tile([128, 1], dt.float32)
                nc.vector.reciprocal(t_rne[:], t_ne[:])
                t_cst = ep.tile([128, KC], dt.float32)
                nc.vector.tensor_scalar(t_cst[:], t_cse[:], t_rne[:], None,
                                        mybir.AluOpType.mult)
                t_cssm = ep.tile([128, KC], dt.float32)
                nc.vector.tensor_scalar(t_cssm[:], t_cst[:], t_nb[:], None,
                                        mybir.AluOpType.mult)

                # embed_normalized = ean * (1 / cs_smoothed)
                t_rcs = ep.tile([128, KC], dt.float32)
                nc.vector.reciprocal(t_rcs[:], t_cssm[:])
                t_enr = ep.tile([128, KC, D], dt.float32)
                for c in range(KC):
                    nc.vector.tensor_scalar(t_enr[:, c, :], t_ean[:, c, :],
                                            t_rcs[:, c:c + 1], None,
                                            mybir.AluOpType.mult)
                nc.sync.dma_start(o_enr[:], t_enr[:].rearrange("p c d -> p (c d)"))

                # ---- quantize gather ----
                # wrapped idx: W[q, c*128+u] = idxf[u, c*16+q] via 16 PE transposes
                t_wrep = ep.tile([128, NL // 16], dt.int16)
                t_wf = ep.tile([16, NL // 16], dt.float32)
                for c in range(16):
                    p_w = epps.tile([16, 128], dt.float32)
                    nc.tensor.transpose(p_w[:], t_idxf[:, c * 16:(c + 1) * 16],
                                        t_ident[:])
                    nc.scalar.copy(t_wf[:, c * 128:(c + 1) * 128], p_w[:])
                nc.vector.tensor_copy(t_wrep[0:16, :], t_wf[:])
                for g in range(1, 8):
                    nc.sync.dma_start(t_wrep[g * 16:(g + 1) * 16, :], t_wrep[0:16, :])

                if DO_GATHER:
                    t_g = ep.tile([128, TPP, D], dt.float32)
                    gc = GATHER_CHUNK
                    for g0 in range(0, NL, gc):
                        nc.gpsimd.dma_gather(
                            out_ap=t_g[:, g0 // 128:(g0 + gc) // 128, :],
                            in_ap=d_etbl[:],
                            idxs_ap=t_wrep[:, g0 // 16:(g0 + gc) // 16],
                            num_idxs=gc, num_idxs_reg=gc, elem_size=D,
                            single_packet=False)
                    nc.sync.dma_start(o_q[:], t_g[:].rearrange("p j d -> p (j d)"))

    nc.is_finalized() or nc.finalize()
    return nc


def _gather_unscramble_idx():
    """row index n handled by gather output cell [P, J]."""
    P_, J_ = np.meshgrid(np.arange(128), np.arange(TPP), indexing="ij")
    return ((J_ % 16) * 8 + P_ // 16) * TPP + (J_ // 16) * 16 + (P_ % 16)


def kernel(x, embed, cluster_size, embed_avg):
    x = np.ascontiguousarray(np.asarray(x, dtype=np.float32))
    embed = np.ascontiguousarray(np.asarray(embed, dtype=np.float32))
    cluster_size = np.asarray(cluster_size, dtype=np.float32)
    embed_avg = np.ascontiguousarray(np.asarray(embed_avg, dtype=np.float32))

    if "nc" not in _cached:
        _cached["nc"] = _build_program()
    nc = _cached["nc"]

    e2 = np.sum(embed.astype(np.float32) * embed, axis=1, dtype=np.float32)
    consts = {
        "e2T": np.ascontiguousarray((2.0 * embed).T),
        "e2neg": np.tile(-e2[None, :], (128, 1)).astype(np.float32),
        "iota": np.tile(np.arange(K, dtype=np.float32)[None, :], (128, 1)),
        "ident": np.eye(128, dtype=np.float32),
        "etbl": embed,
        "cs_r": np.ascontiguousarray(cluster_size.reshape(KC, 128).T),
        "ea_r": np.ascontiguousarray(
            embed_avg.reshape(KC, 128, D).transpose(1, 0, 2).reshape(128, KC * D)),
    }
    in_maps = []
    for c in range(NCORES):
        shard = np.ascontiguousarray(x[c * NL:(c + 1) * NL])
        in_maps.append({"x_loc": shard, **consts})

    _cached["in_maps"] = in_maps
    r = run_bass_kernel_spmd(nc, in_maps, list(range(NCORES)))
    res = r.results
    _cached["last_results"] = res
    _cached["exec_time_ns"] = r.exec_time_ns
    _cached["profile_json"] = r.profile_json

    n_of = _gather_unscramble_idx()
    quantize = np.empty((N, D), np.float32)
    embed_ind = np.empty((N,), np.int32)
    for c in range(NCORES):
        r = res[c]
        ei = np.asarray(r["ei_r"], np.int32)          # [128, TPP] at [p, t]
        embed_ind[c * NL:(c + 1) * NL] = ei.reshape(NL)
        q3 = np.asarray(r["q_r"], np.float32).reshape(128, TPP, D)
        qs = np.empty((NL, D), np.float32)
        qs[n_of.ravel()] = q3.reshape(NL, D)
        quantize[c * NL:(c + 1) * NL] = qs

    r0 = res[0]
    cluster_size_new = np.asarray(r0["csn_r"], np.float32).T.ravel().copy()
    embed_avg_new = np.ascontiguousarray(
        np.asarray(r0["ean_r"], np.float32).reshape(128, KC, D)
        .transpose(1, 0, 2).reshape(K, D))
    embed_normalized = np.ascontiguousarray(
        np.asarray(r0["enr_r"], np.float32).reshape(128, KC, D)
        .transpose(1, 0, 2).reshape(K, D))

    return quantize, embed_ind, cluster_size_new, embed_avg_new, embed_normalized


def bench(reps=10):
    """Time the compiled NEFF with device-resident inputs (median wall ns/run)."""
    import time
    import jax
    import jax.numpy as jnp
    from jax.sharding import Mesh, PartitionSpec
    from jax.experimental.shard_map import shard_map
    from concourse import bass2jax as b2j
    import concourse.mybir as mb

    nc = _cached["nc"]
    in_maps = _cached["in_maps"]
    b2j.install_neuronx_cc_hook()
    partition_name = nc.partition_id_tensor.name if nc.partition_id_tensor else None
    in_names, out_names, out_avals, zero_outs = [], [], [], []
    for alloc in nc.m.functions[0].allocations:
        if not isinstance(alloc, mybir.MemoryLocationSet):
            continue
        name = alloc.memorylocations[0].name
        if alloc.kind == "ExternalInput":
            if name != partition_name:
                in_names.append(name)
        elif alloc.kind == "ExternalOutput":
            out_names.append(name)
            shape = tuple(alloc.tensor_shape)
            dtp = mb.dt.np(alloc.dtype)
            out_avals.append(jax.core.ShapedArray(shape, dtp))
            zero_outs.append(np.zeros(shape, dtp))
    n_params = len(in_names)
    all_in_names = in_names + out_names + ([partition_name] if partition_name else [])

    def _body(*args):
        operands = list(args)
        if partition_name is not None:
            operands.append(b2j.partition_id_tensor())
        return tuple(b2j._bass_exec_p.bind(
            *operands, out_avals=tuple(out_avals), in_names=tuple(all_in_names),
            out_names=tuple(out_names), lowering_input_output_aliases=(),
            sim_require_finite=True, sim_require_nnan=True, nc=nc))

    def _body_k(k):
        def f(*args):
            ins = list(args[:n_params])
            zouts = list(args[n_params:])
            for _ in range(k):
                operands = ins + zouts
                if partition_name is not None:
                    operands.append(b2j.partition_id_tensor())
                zouts = list(b2j._bass_exec_p.bind(
                    *operands, out_avals=tuple(out_avals),
                    in_names=tuple(all_in_names), out_names=tuple(out_names),
                    lowering_input_output_aliases=(),
                    sim_require_finite=True, sim_require_nnan=True, nc=nc))
            return tuple(zouts)
        return f

    devices = jax.devices()[:NCORES]
    mesh = Mesh(np.asarray(devices), ("core",))
    nin = n_params + len(out_names)
    def make_fn(k):
        return jax.jit(shard_map(_body_k(k), mesh=mesh,
                       in_specs=(PartitionSpec("core"),) * nin,
                       out_specs=(PartitionSpec("core"),) * len(out_names),
                       check_rep=False))
    fn = make_fn(1)
    from jax.sharding import NamedSharding
    sh = NamedSharding(mesh, PartitionSpec("core"))
    concat_in = [np.concatenate([in_maps[c][nm] for c in range(NCORES)], axis=0)
                 for nm in in_names]
    concat_in += [np.concatenate([z] * NCORES, axis=0) for z in zero_outs]
    dev_in = [jax.device_put(a, sh) for a in concat_in]
    def timed(f, r):
        jax.block_until_ready(f(*dev_in))
        ts = []
        for _ in range(r):
            t0 = time.perf_counter_ns()
            jax.block_until_ready(f(*dev_in))
            ts.append(time.perf_counter_ns() - t0)
        ts.sort()
        return ts
    t1 = timed(fn, reps)
    fn9 = make_fn(9)
    t9 = timed(fn9, reps)
    per_exec = (t9[len(t9) // 2] - t1[len(t1) // 2]) / 8.0
    return {"median_ns": t1[len(t1) // 2], "min_ns": t1[0],
            "k9_median_ns": t9[len(t9) // 2], "per_exec_ns": per_exec,
            "all1": t1, "all9": t9}


# revision 25
# speedup vs baseline: 1.1519x; 1.1519x over previous
"""TRN2 Bass kernel for nn_EuclideanCodebook (VQ codebook with EMA update).

Strategy (data-parallel over N, 8 NeuronCores):
 - shard x [262144, 64] row-wise -> 32768 rows/core; replicate codebook consts
 - per 128-row tile: PE computes s = x @ (-2 e)^T (fp32, exact reference
   rounding order: dist = (x2 - 2s) + e2 reproduced via ACT bias + DVE add,
   negated end-to-end so DVE max8/max_index give argmin w/ first-tie like jnp)
 - onehot(argmin) built on GPSIMD; PE scatter-matmul accumulates
   [embed_sum | bins] in PSUM across all tiles
 - AllReduce bins/embed_sum across the 8 cores (gpsimd collective), then the
   EMA + Laplace-smoothing epilogue on-device
 - quantize = embed[embed_ind] via one dma_gather (idx relayout through 16
   PE transposes into the 16-partition-wrapped int16 index layout)
"""
import sys
sys.path.insert(0, '/opt/trn_rl_repo')
import numpy as np
import concourse.bass as bass
from concourse import bacc
import concourse.mybir as mybir
import concourse.tile as tile
from concourse.bass_utils import run_bass_kernel_spmd

dt = mybir.dt

N, D, K = 262144, 64, 1024
NCORES = 8
NL = N // NCORES          # 32768 rows per core
TPP = NL // 128           # 256 rows per partition (= tiles per core)
T = TPP                   # 256 tiles of 128 rows
KC = K // 128             # 8 onehot chunks
DECAY = 0.1
EPS = 1e-5

_cached = {}

# feature flags (bisection/testing)
DO_SCATTER = True
DO_CC = True
DO_GATHER = True
GATHER_CHUNK = 4096       # rows per dma_gather call
OH_ON_DVE = False         # onehot on DVE instead of GPSIMD
LINEARIZE = False
DEBUG_OHSUM = False


def _build_program():
    nc = bacc.Bacc("TRN2", target_bir_lowering=False, debug=False)

    # inputs
    d_x = nc.dram_tensor("x_loc", [NL, D], dt.float32, kind="ExternalInput")
    d_e2T = nc.dram_tensor("e2T", [D, K], dt.float32, kind="ExternalInput")      # (2*embed)^T
    d_e2neg = nc.dram_tensor("e2neg", [128, K], dt.float32, kind="ExternalInput")  # -|e|^2 replicated
    d_iota = nc.dram_tensor("iota", [128, K], dt.float32, kind="ExternalInput")
    d_ident = nc.dram_tensor("ident", [128, 128], dt.float32, kind="ExternalInput")
    d_etbl = nc.dram_tensor("etbl", [K, D], dt.float32, kind="ExternalInput")    # embed for gather
    d_cs = nc.dram_tensor("cs_r", [128, KC], dt.float32, kind="ExternalInput")   # cluster_size [p,c]
    d_ea = nc.dram_tensor("ea_r", [128, KC * D], dt.float32, kind="ExternalInput")  # embed_avg [p,c,:]

    # outputs
    o_q = nc.dram_tensor("q_r", [128, TPP * D], dt.float32, kind="ExternalOutput")
    o_ei = nc.dram_tensor("ei_r", [128, TPP], dt.int32, kind="ExternalOutput")
    o_csn = nc.dram_tensor("csn_r", [128, KC], dt.float32, kind="ExternalOutput")
    o_ean = nc.dram_tensor("ean_r", [128, KC * D], dt.float32, kind="ExternalOutput")
    o_enr = nc.dram_tensor("enr_r", [128, KC * D], dt.float32, kind="ExternalOutput")
    o_dbg = (nc.dram_tensor("dbg_r", [128, TPP], dt.float32, kind="ExternalOutput")
             if DEBUG_OHSUM else None)

    # collective bounce buffers
    cc_in = nc.dram_tensor("cc_in", [128, KC * (D + 1)], dt.float32)
    cc_out = nc.dram_tensor("cc_out", [128, KC * (D + 1)], dt.float32, addr_space="Shared")

    x_view = d_x.rearrange("(p t) d -> p t d", p=128)     # row n = p*TPP + t

    with tile.TileContext(nc, linearize=LINEARIZE) as tc:
        with tc.tile_pool(name="const", bufs=1) as cpool, \
             tc.tile_pool(name="xall", bufs=1) as xpool, \
             tc.tile_pool(name="acc", bufs=1) as apool, \
             tc.tile_pool(name="pacc", bufs=1, space="PSUM") as ppacc:

            # ---- resident tiles ----
            t_e2T = cpool.tile([D, K], dt.float32)
            nc.sync.dma_start(t_e2T[:], d_e2T[:])
            t_e2neg = cpool.tile([128, K], dt.float32)
            nc.sync.dma_start(t_e2neg[:], d_e2neg[:])
            t_iota = cpool.tile([128, K], dt.float32)
            nc.sync.dma_start(t_iota[:], d_iota[:])
            t_ident = cpool.tile([128, 128], dt.float32)
            nc.sync.dma_start(t_ident[:], d_ident[:])

            # x resident: [128, T, D+1] with a ones column at d=D
            t_xall = xpool.tile([128, T, D + 1], dt.float32)
            nc.sync.dma_start(t_xall[:, :, 0:D], x_view[:])
            nc.vector.memset(t_xall[:, :, D:D + 1], 1.0)

            # index accumulator (fp32, exact ints)
            t_idxf = apool.tile([128, T], dt.float32)
            t_ohsum = apool.tile([128, T], dt.float32, name="t_ohsum") if DEBUG_OHSUM else None

            # [embed_sum | bins] PSUM accumulator: chunk c at [:, c, 0:65]
            p_es = ppacc.tile([128, KC, 128], dt.float32)

            with tc.tile_pool(name="sb", bufs=4) as pool, \
                 tc.tile_pool(name="ps", bufs=2, space="PSUM") as pps, \
                 tc.tile_pool(name="psT", bufs=2, space="PSUM") as ppsT:
                for t in range(T):
                    xt = t_xall[:, t, 0:D]
                    rhs65 = t_xall[:, t, :]

                    # xT via PE transpose
                    p_xT = ppsT.tile([D, 128], dt.float32)
                    nc.tensor.transpose(p_xT[:], xt, t_ident[:])
                    t_xT = pool.tile([D, 128], dt.float32, tag="xT")
                    nc.scalar.copy(t_xT[:], p_xT[:])

                    # x2 via ACT square + accumulate, then negate
                    t_sq = pool.tile([128, D], dt.float32, tag="sq")
                    t_x2 = pool.tile([128, 1], dt.float32, tag="x2")
                    nc.scalar.activation(t_sq[:], xt,
                                         mybir.ActivationFunctionType.Square,
                                         accum_out=t_x2[:])
                    t_x2n = pool.tile([128, 1], dt.float32, tag="x2n")
                    nc.vector.tensor_scalar_mul(t_x2n[:], t_x2[:], -1.0)

                    # s' = x @ (2e)^T  [128, 1024] fp32 (2 PSUM banks)
                    p_s = pps.tile([128, K], dt.float32)
                    nc.tensor.matmul(p_s[:, 0:512], t_xT[:], t_e2T[:, 0:512],
                                     start=True, stop=True)
                    nc.tensor.matmul(p_s[:, 512:1024], t_xT[:], t_e2T[:, 512:1024],
                                     start=True, stop=True)

                    # tt = fl(s' - x2) = -fl(x2 - 2s)
                    t_t = pool.tile([128, K], dt.float32, tag="t")
                    nc.scalar.activation(t_t[:], p_s[:],
                                         mybir.ActivationFunctionType.Identity,
                                         bias=t_x2n[:], scale=1.0)
                    # d' = fl(tt - e2) = -dist  (split DVE/GPSIMD to balance)
                    t_d = pool.tile([128, K], dt.float32, tag="d")
                    SPL = 320
                    nc.vector.tensor_tensor(t_d[:, 0:SPL], t_t[:, 0:SPL],
                                            t_e2neg[:, 0:SPL], mybir.AluOpType.add)
                    nc.gpsimd.tensor_tensor(t_d[:, SPL:], t_t[:, SPL:],
                                            t_e2neg[:, SPL:], mybir.AluOpType.add)

                    # argmin via max8 + max_index (first-tie like jnp.argmin)
                    t_m8 = pool.tile([128, 8], dt.float32, tag="m8")
                    nc.vector.max(out=t_m8[:], in_=t_d[:])
                    t_i8 = pool.tile([128, 8], dt.uint16, tag="i8")
                    nc.vector.max_index(t_i8[:], t_m8[:], t_d[:])
                    nc.vector.tensor_copy(t_idxf[:, t:t + 1], t_i8[:, 0:1])

                    # onehot
                    t_oh = pool.tile([128, K], dt.float32, tag="oh")
                    oh_eng = nc.vector if OH_ON_DVE else nc.gpsimd
                    oh_eng.tensor_scalar(t_oh[:], t_iota[:],
                                         t_idxf[:, t:t + 1], None,
                                         mybir.AluOpType.is_equal)

                    if DEBUG_OHSUM:
                        nc.vector.tensor_reduce(t_ohsum[:, t:t + 1], t_oh[:],
                                                mybir.AxisListType.X,
                                                mybir.AluOpType.add)

                    # scatter-accumulate [embed_sum | bins]
                    if DO_SCATTER:
                        # start=True clears has_written for the WHOLE 2KB bank,
                        # so only the first chunk of each bank may set it.
                        for c in range(KC):
                            nc.tensor.matmul(p_es[:, c, 0:D + 1],
                                             t_oh[:, c * 128:(c + 1) * 128], rhs65,
                                             start=(t == 0 and c % 4 == 0),
                                             stop=(t == T - 1),
                                             skip_group_check=True)

            # ================= epilogue =================
            with tc.tile_pool(name="ep", bufs=1) as ep, \
                 tc.tile_pool(name="eps", bufs=1, space="PSUM") as epps:
                # embed_sum partial -> SBUF -> AllReduce
                t_es = ep.tile([128, KC, D + 1], dt.float32)
                if DO_SCATTER:
                    nc.scalar.copy(t_es[:], p_es[:, :, 0:D + 1])
                else:
                    nc.vector.memset(t_es[:], 1.0)
                t_esg = ep.tile([128, KC, D + 1], dt.float32)
                if DO_CC:
                    nc.sync.dma_start(cc_in[:], t_es[:].rearrange("p c k -> p (c k)"))
                    nc.gpsimd.collective_compute(
                        "AllReduce", mybir.AluOpType.add,
                        replica_groups=[list(range(NCORES))],
                        ins=[cc_in[:]], outs=[cc_out[:]])
                    nc.sync.dma_start(t_esg[:].rearrange("p c k -> p (c k)"), cc_out[:])
                else:
                    nc.vector.tensor_copy(t_esg[:], t_es[:])

                if DEBUG_OHSUM:
                    nc.sync.dma_start(o_dbg[:], t_ohsum[:])
                # embed_ind int32 out
                t_ei = ep.tile([128, T], dt.int32)
                nc.vector.tensor_copy(t_ei[:], t_idxf[:])
                nc.sync.dma_start(o_ei[:], t_ei[:])

                # ---- EMA update ----
                t_cs = ep.tile([128, KC], dt.float32)
                nc.sync.dma_start(t_cs[:], d_cs[:])
                t_ea = ep.tile([128, KC, D], dt.float32)
                nc.sync.dma_start(t_ea[:].rearrange("p c d -> p (c d)"), d_ea[:])

                bins = t_esg[:, :, D:D + 1]          # [128, KC, 1]
                esum = t_esg[:, :, 0:D]              # [128, KC, D]

                # cluster_size_new = cs*DECAY + bins*(1-DECAY)
                t_csd = ep.tile([128, KC], dt.float32)
                nc.vector.tensor_scalar_mul(t_csd[:], t_cs[:], DECAY)
                t_bnd = ep.tile([128, KC], dt.float32)
                nc.vector.tensor_scalar_mul(
                    t_bnd[:], bins.rearrange("p c one -> p (c one)"), 1.0 - DECAY)
                t_csn = ep.tile([128, KC], dt.float32)
                nc.vector.tensor_tensor(t_csn[:], t_csd[:], t_bnd[:],
                                        mybir.AluOpType.add)
                nc.sync.dma_start(o_csn[:], t_csn[:])

                # embed_avg_new = ea*DECAY + esum*(1-DECAY)
                t_ead = ep.tile([128, KC, D], dt.float32)
                nc.vector.tensor_scalar_mul(t_ead[:], t_ea[:], DECAY)
                t_esd = ep.tile([128, KC, D], dt.float32)
                nc.vector.tensor_scalar_mul(t_esd[:], esum, 1.0 - DECAY)
                t_ean = ep.tile([128, KC, D], dt.float32)
                nc.vector.tensor_tensor(t_ean[:], t_ead[:], t_esd[:],
                                        mybir.AluOpType.add)
                nc.sync.dma_start(o_ean[:], t_ean[:].rearrange("p c d -> p (c d)"))

                # n = sum(cluster_size_new) -> broadcast to all partitions
                t_ones = ep.tile([128, 1], dt.float32)
                nc.vector.memset(t_ones[:], 1.0)
                t_csp = ep.tile([128, 1], dt.float32)
                nc.vector.tensor_reduce(t_csp[:], t_csn[:], mybir.AxisListType.X,
                                        mybir.AluOpType.add)
                p_n = epps.tile([1, 1], dt.float32)
                nc.tensor.matmul(p_n[:], t_csp[:], t_ones[:],
                                 start=True, stop=True)
                t_n1 = ep.tile([1, 1], dt.float32)
                nc.scalar.copy(t_n1[:], p_n[:])
                t_ones_row = ep.tile([1, 128], dt.float32)
                nc.vector.memset(t_ones_row[:], 1.0)
                p_nb = epps.tile([128, 1], dt.float32)
                nc.tensor.matmul(p_nb[:], t_ones_row[:], t_n1[:],
                                 start=True, stop=True)
                t_nb = ep.tile([128, 1], dt.float32)
                nc.scalar.copy(t_nb[:], p_nb[:])

                # cs_smoothed = (csn + EPS) / (n + K*EPS) * n
                t_cse = ep.tile([128, KC], dt.float32)
                nc.vector.tensor_scalar_add(t_cse[:], t_csn[:], EPS)
                t_ne = ep.tile([128, 1], dt.float32)
                nc.vector.tensor_scalar_add(t_ne[:], t_nb[:], float(K * EPS))
                t_rne = ep.tile([128, 1], dt.float32)
                nc.vector.reciprocal(t_rne[:], t_ne[:])
                t_cst = ep.tile([128, KC], dt.float32)
                nc.vector.tensor_scalar(t_cst[:], t_cse[:], t_rne[:], None,
                                        mybir.AluOpType.mult)
                t_cssm = ep.tile([128, KC], dt.float32)
                nc.vector.tensor_scalar(t_cssm[:], t_cst[:], t_nb[:], None,
                                        mybir.AluOpType.mult)

                # embed_normalized = ean * (1 / cs_smoothed)
                t_rcs = ep.tile([128, KC], dt.float32)
                nc.vector.reciprocal(t_rcs[:], t_cssm[:])
                t_enr = ep.tile([128, KC, D], dt.float32)
                for c in range(KC):
                    nc.vector.tensor_scalar(t_enr[:, c, :], t_ean[:, c, :],
                                            t_rcs[:, c:c + 1], None,
                                            mybir.AluOpType.mult)
                nc.sync.dma_start(o_enr[:], t_enr[:].rearrange("p c d -> p (c d)"))

                # ---- quantize gather ----
                # wrapped idx: W[q, c*128+u] = idxf[u, c*16+q] via 16 PE transposes
                t_wrep = ep.tile([128, NL // 16], dt.int16)
                t_wf = ep.tile([16, NL // 16], dt.float32)
                for c in range(16):
                    p_w = epps.tile([16, 128], dt.float32)
                    nc.tensor.transpose(p_w[:], t_idxf[:, c * 16:(c + 1) * 16],
                                        t_ident[:])
                    nc.scalar.copy(t_wf[:, c * 128:(c + 1) * 128], p_w[:])
                nc.vector.tensor_copy(t_wrep[0:16, :], t_wf[:])
                for g in range(1, 8):
                    nc.sync.dma_start(t_wrep[g * 16:(g + 1) * 16, :], t_wrep[0:16, :])

                if DO_GATHER:
                    t_g = ep.tile([128, TPP, D], dt.float32)
                    gc = GATHER_CHUNK
                    for g0 in range(0, NL, gc):
                        nc.gpsimd.dma_gather(
                            out_ap=t_g[:, g0 // 128:(g0 + gc) // 128, :],
                            in_ap=d_etbl[:],
                            idxs_ap=t_wrep[:, g0 // 16:(g0 + gc) // 16],
                            num_idxs=gc, num_idxs_reg=gc, elem_size=D,
                            single_packet=False)
                    nc.sync.dma_start(o_q[:], t_g[:].rearrange("p j d -> p (j d)"))

    nc.is_finalized() or nc.finalize()
    return nc


def _gather_unscramble_idx():
    """row index n handled by gather output cell [P, J]."""
    P_, J_ = np.meshgrid(np.arange(128), np.arange(TPP), indexing="ij")
    return ((J_ % 16) * 8 + P_ // 16) * TPP + (J_ // 16) * 16 + (P_ % 16)


def kernel(x, embed, cluster_size, embed_avg):
    x = np.ascontiguousarray(np.asarray(x, dtype=np.float32))
    embed = np.ascontiguousarray(np.asarray(embed, dtype=np.float32))
    cluster_size = np.asarray(cluster_size, dtype=np.float32)
    embed_avg = np.ascontiguousarray(np.asarray(embed_avg, dtype=np.float32))

    if "nc" not in _cached:
        _cached["nc"] = _build_program()
    nc = _cached["nc"]

    e2 = np.sum(embed.astype(np.float32) * embed, axis=1, dtype=np.float32)
    consts = {
        "e2T": np.ascontiguousarray((2.0 * embed).T),
        "e2neg": np.tile(-e2[None, :], (128, 1)).astype(np.float32),
        "iota": np.tile(np.arange(K, dtype=np.float32)[None, :], (128, 1)),
        "ident": np.eye(128, dtype=np.float32),
        "etbl": embed,
        "cs_r": np.ascontiguousarray(cluster_size.reshape(KC, 128).T),
        "ea_r": np.ascontiguousarray(
            embed_avg.reshape(KC, 128, D).transpose(1, 0, 2).reshape(128, KC * D)),
    }
    in_maps = []
    for c in range(NCORES):
        shard = np.ascontiguousarray(x[c * NL:(c + 1) * NL])
        in_maps.append({"x_loc": shard, **consts})

    _cached["in_maps"] = in_maps
    r = run_bass_kernel_spmd(nc, in_maps, list(range(NCORES)))
    res = r.results
    _cached["last_results"] = res
    _cached["exec_time_ns"] = r.exec_time_ns
    _cached["profile_json"] = r.profile_json

    n_of = _gather_unscramble_idx()
    quantize = np.empty((N, D), np.float32)
    embed_ind = np.empty((N,), np.int32)
    for c in range(NCORES):
        r = res[c]
        ei = np.asarray(r["ei_r"], np.int32)          # [128, TPP] at [p, t]
        embed_ind[c * NL:(c + 1) * NL] = ei.reshape(NL)
        q3 = np.asarray(r["q_r"], np.float32).reshape(128, TPP, D)
        qs = np.empty((NL, D), np.float32)
        qs[n_of.ravel()] = q3.reshape(NL, D)
        quantize[c * NL:(c + 1) * NL] = qs

    r0 = res[0]
    cluster_size_new = np.asarray(r0["csn_r"], np.float32).T.ravel().copy()
    embed_avg_new = np.ascontiguousarray(
        np.asarray(r0["ean_r"], np.float32).reshape(128, KC, D)
        .transpose(1, 0, 2).reshape(K, D))
    embed_normalized = np.ascontiguousarray(
        np.asarray(r0["enr_r"], np.float32).reshape(128, KC, D)
        .transpose(1, 0, 2).reshape(K, D))

    return quantize, embed_ind, cluster_size_new, embed_avg_new, embed_normalized


def bench(reps=10):
    """Time the compiled NEFF with device-resident inputs (median wall ns/run)."""
    import time
    import jax
    import jax.numpy as jnp
    from jax.sharding import Mesh, PartitionSpec
    from jax.experimental.shard_map import shard_map
    from concourse import bass2jax as b2j
    import concourse.mybir as mb

    nc = _cached["nc"]
    in_maps = _cached["in_maps"]
    b2j.install_neuronx_cc_hook()
    partition_name = nc.partition_id_tensor.name if nc.partition_id_tensor else None
    in_names, out_names, out_avals, zero_outs = [], [], [], []
    for alloc in nc.m.functions[0].allocations:
        if not isinstance(alloc, mybir.MemoryLocationSet):
            continue
        name = alloc.memorylocations[0].name
        if alloc.kind == "ExternalInput":
            if name != partition_name:
                in_names.append(name)
        elif alloc.kind == "ExternalOutput":
            out_names.append(name)
            shape = tuple(alloc.tensor_shape)
            dtp = mb.dt.np(alloc.dtype)
            out_avals.append(jax.core.ShapedArray(shape, dtp))
            zero_outs.append(np.zeros(shape, dtp))
    n_params = len(in_names)
    all_in_names = in_names + out_names + ([partition_name] if partition_name else [])

    def _body(*args):
        operands = list(args)
        if partition_name is not None:
            operands.append(b2j.partition_id_tensor())
        return tuple(b2j._bass_exec_p.bind(
            *operands, out_avals=tuple(out_avals), in_names=tuple(all_in_names),
            out_names=tuple(out_names), lowering_input_output_aliases=(),
            sim_require_finite=True, sim_require_nnan=True, nc=nc))

    def _body_k(k):
        def f(*args):
            ins = list(args[:n_params])
            zouts = list(args[n_params:])
            for _ in range(k):
                operands = ins + zouts
                if partition_name is not None:
                    operands.append(b2j.partition_id_tensor())
                zouts = list(b2j._bass_exec_p.bind(
                    *operands, out_avals=tuple(out_avals),
                    in_names=tuple(all_in_names), out_names=tuple(out_names),
                    lowering_input_output_aliases=(),
                    sim_require_finite=True, sim_require_nnan=True, nc=nc))
            return tuple(zouts)
        return f

    devices = jax.devices()[:NCORES]
    mesh = Mesh(np.asarray(devices), ("core",))
    nin = n_params + len(out_names)
    def make_fn(k):
        return jax.jit(shard_map(_body_k(k), mesh=mesh,
                       in_specs=(PartitionSpec("core"),) * nin,
                       out_specs=(PartitionSpec("core"),) * len(out_names),
                       check_rep=False))
    fn = make_fn(1)
    from jax.sharding import NamedSharding
    sh = NamedSharding(mesh, PartitionSpec("core"))
    concat_in = [np.concatenate([in_maps[c][nm] for c in range(NCORES)], axis=0)
                 for nm in in_names]
    concat_in += [np.concatenate([z] * NCORES, axis=0) for z in zero_outs]
    dev_in = [jax.device_put(a, sh) for a in concat_in]
    def timed(f, r):
        jax.block_until_ready(f(*dev_in))
        ts = []
        for _ in range(r):
            t0 = time.perf_counter_ns()
            jax.block_until_ready(f(*dev_in))
            ts.append(time.perf_counter_ns() - t0)
        ts.sort()
        return ts
    t1 = timed(fn, reps)
    fn9 = make_fn(9)
    t9 = timed(fn9, reps)
    per_exec = (t9[len(t9) // 2] - t1[len(t1) // 2]) / 8.0
    return {"median_ns": t1[len(t1) // 2], "min_ns": t1[0],
            "k9_median_ns": t9[len(t9) // 2], "per_exec_ns": per_exec,
            "all1": t1, "all9": t9}
